# revision 37
# baseline (speedup 1.0000x reference)
"""Trainium2 Bass kernel for a 2-layer GCN forward pass (8 NeuronCores).

    h      = relu(spmm(A, x @ W1) + b1)
    out    = softmax(spmm(A, h @ W2) + b2)   with spmm(A, h) @ W2 == spmm(A, h @ W2)

Strategy (graph/data parallel over 8 cores):
  K1: node-sharded dense matmul  support = x @ W1            (per-core rows)
  host: assemble full `support` gather table from the 8 shards (pure movement)
  K2: dst-sharded spmm + bias + relu -> (h @ W2) shard       (per-core rows)
  host: assemble full `h @ W2` table (padded rows)
  K3: dst-sharded spmm(16 cols) + b2 -> softmax -> out shard

spmm per core (dst tiles of 128 rows, chunks of 5 tiles):
  * host BIN-PACKS destination nodes into tiles against per-(tile, block)
    slot QUOTAS (multiples of 128, shared across cores, sized from the
    worst core +6%): sparse src-blocks are concentrated into the LAST
    tiles (small final chunks -> short kernel tail), dense blocks get a
    balanced 640/768-style split so per-tile totals match the ~2048-edge
    supply of a 128-row tile. Slot padding ~6% vs 14% for naive packing.
  * per (chunk, src-block) one `dma_gather` (int16 indices limit the table
    view to 32768 rows -> 4 blocks) fetches 256B rows from the HBM table.
    Calls round-robin the 4 SWDGE queues BY CALL COUNT (ring drain of call
    N-1 overlaps Q7 descriptor-gen of call N) and each call stays near the
    ~2.5-3k index sweet spot; larger calls hit SWDGE ring backpressure.
    Q7 descriptor generation (~2.6ns/idx, engine-serial) is the kernel's
    critical path; idx/dst/val tables are split per chunk so the first
    gather starts immediately.
  * edge values fold into the gathered rows with one broadcast multiply
    per (chunk, block) that also casts to fp16 (pad slots have val=0 ->
    contribute 0). Single fp16 pass: rel_err ~9e-3 vs the 2e-2 gate (the
    hi+lo fp16 trick is not needed; fp32 K1 matmuls ARE needed - bf16
    there alone costs 6.6e-2).
  * segment-sum as accumulating PE matmuls. Layer 1 accumulates the
    TRANSPOSED aggregate psum[64 feats, 128 rows] = ghi.T @ S so the
    per-feature b1 + relu fuse into the single ACT evacuation (per-
    partition bias) and the result feeds W2 directly without a transpose;
    the hw2 = relu(agg+b1) @ W2 [*,16] shard is the layer-2 table (A(hW2)
    == (Ah)W2). Layer 2 aggregates psum[128 rows, 16] then fused
    bias/softmax. All S masks of a half-chunk are built by ONE DVE
    tensor_tensor(is_equal) slab (f32 in, fp16 0/1 out; broadcast-pattern
    ops run at ~1 elem/lane/cycle regardless of dtype).
  * K1 is weight-stationary (x^T tiles as rhs, W1 slabs as lhsT, kb outer)
    so LDWEIGHTS fires 4x per 2048-node batch instead of per-matmul.
"""
import os
import sys
import time

for _p in ("/opt/trn_rl_repo", "/opt/pypackages"):
    if _p not in sys.path:
        sys.path.append(_p)

import numpy as np
from concourse import bacc, mybir, tile, bass_utils

F32 = mybir.dt.float32
F16 = mybir.dt.float16
BF16 = mybir.dt.bfloat16
I16 = mybir.dt.int16
AX = mybir.AxisListType.X
EQ = mybir.AluOpType.is_equal
MUL = mybir.AluOpType.mult
ADD = mybir.AluOpType.add
EXP = mybir.ActivationFunctionType.Exp
RELU = mybir.ActivationFunctionType.Relu
CPY = mybir.ActivationFunctionType.Copy

P = 128


class Cfg:
    def __init__(self, n_nodes=100000, f_in=512, hidden=64, n_class=16,
                 n_cores=8, chunk_tiles=6, blk=32768):
        self.n_nodes, self.f_in, self.hidden, self.n_class = n_nodes, f_in, hidden, n_class
        self.n_cores, self.chunk_tiles, self.blk = n_cores, chunk_tiles, blk
        assert n_nodes % n_cores == 0
        self.npc = n_nodes // n_cores
        self.tpc = -(-self.npc // P)
        self.rows_pad = self.tpc * P
        self.nblk = -(-n_nodes // blk)
        self.table_rows = self.nblk * blk
        assert f_in % P == 0
        self.kb = f_in // P
        self.n_chunks = -(-self.tpc // chunk_tiles)


def _make_quotas(cfg, deg_all):
    """Per-(tile, block) slot quotas in multiples of 128, shared across all
    cores, sized from the worst core's per-block edge totals.

    Low-traffic blocks are concentrated into few tiles (the rest get quota
    0 -> no gather groups at all); high-traffic blocks get an even
    640/768-style split."""
    tpc, nblk = cfg.tpc, cfg.nblk
    btot = deg_all.sum(1)                     # [ncores, nblk]
    bmax = btot.max(0)                        # worst core per block
    # rows needed to host every node that has edges in block b
    nwith = (deg_all > 0).sum(1).max(0)       # [nblk]
    quotas = np.zeros((tpc, nblk), np.int64)
    # base quota per block, then spread the remainder in +128 steps onto
    # the tiles with the smallest running total so per-tile edge totals
    # stay near the ~128*avg_degree supply a full tile can deliver
    order_b = np.argsort(-bmax)
    for b in order_b:
        tot_q = (int(bmax[b] * 1.06) + 127) // 128 * 128
        if tot_q == 0:
            continue
        if tot_q <= tpc * P // 2:
            # concentrate into the LAST tiles (row-feasible): the final
            # chunks then carry fewer groups, shrinking the kernel tail
            # that runs after the last gather
            k = max(tot_q // P, (int(nwith[b]) + 110) // 111)
            k = min(k, tpc)
            per = (-(-tot_q // k) + 127) // 128 * 128 if k else 0
            quotas[tpc - k:, b] = per
        else:
            q_lo = tot_q // tpc // P * P
            quotas[:, b] = q_lo
            n_hi = (tot_q - tpc * q_lo + P - 1) // P
            for _ in range(n_hi):
                t = int(np.argmin(quotas.sum(1)))
                quotas[t, b] += P
    return quotas


def _pack_tiles(cfg, deg, quotas):
    """Greedy 4-D bin packing of one core's dst nodes into tiles against
    shared quotas.

    deg: [npc, nblk] per-node per-block in-degree. Returns (tile, row) per
    node. Rows are capped at 128 [hard]; per-(tile,block) quotas [soft]."""
    npc, tpc, nblk = cfg.npc, cfg.tpc, cfg.nblk
    used = np.zeros((tpc, nblk), np.float64)
    q = np.maximum(quotas.astype(np.float64), 1e-9)
    rows = np.zeros(tpc, np.int64)
    t_of = np.zeros(npc, np.int64)
    # scarce blocks first (their quota tiles must host those nodes), then
    # big nodes while there is still room to balance them
    w = 1.0 / np.maximum(quotas.sum(0), 1).astype(np.float64)
    score = (deg * w).sum(1) * 1e6 + deg.sum(1)
    order = np.argsort(-score, kind="stable")
    degf = deg.astype(np.float64)
    for n in order:
        d = deg[n]
        df = degf[n]
        ok = (quotas >= used + d).all(1) & (rows < P)
        # balanced fill: minimize the worst relative load (blocks + rows)
        load = np.maximum(((used + df) / q).max(1), (rows + 1) / P)
        if ok.any():
            load[~ok] = np.inf
            t = int(np.argmin(load))
        else:
            open_ = rows < P
            over = np.maximum(d - (quotas - used), 0).sum(1)
            over[~open_] = np.inf
            t = int(np.argmin(over + 0.001 * load))
        t_of[n] = t
        used[t] += d
        rows[t] += 1
    # stable row numbering within each tile
    r_of = np.zeros(npc, np.int64)
    ordn = np.argsort(t_of, kind="stable")
    tt = t_of[ordn]
    first = np.r_[True, tt[1:] != tt[:-1]]
    starts = np.flatnonzero(first)
    sizes = np.diff(np.r_[starts, npc])
    r_of[ordn] = np.arange(npc) - np.repeat(starts, sizes)
    assert r_of.max() < P
    return t_of, r_of


class Sched:
    """Static (cross-core identical) spmm schedule + per-core slot arrays."""

    def __init__(self, cfg: Cfg, edge_src, edge_dst, edge_val):
        self.cfg = cfg
        ncr, nch, nblk, ct, tpc = (cfg.n_cores, cfg.n_chunks, cfg.nblk,
                                   cfg.chunk_tiles, cfg.tpc)

        core = edge_dst // cfg.npc
        dst_l = edge_dst % cfg.npc
        blk_id = edge_src // cfg.blk

        # per-core node -> (tile, row) packing
        deg = np.zeros((ncr, cfg.npc, nblk), np.int64)
        np.add.at(deg, (core, dst_l, blk_id), 1)
        quotas = _make_quotas(cfg, deg)
        self.t_of = np.zeros((ncr, cfg.npc), np.int64)
        self.r_of = np.zeros((ncr, cfg.npc), np.int64)
        for c in range(ncr):
            self.t_of[c], self.r_of[c] = _pack_tiles(cfg, deg[c], quotas)
        # outrow[c, n_local] = row in the padded shard output
        self.outrow = self.t_of * P + self.r_of

        tl_e = self.t_of[core, dst_l]          # dst tile per edge
        row_e = self.r_of[core, dst_l]         # row within tile per edge
        chunk = tl_e // ct

        order = np.lexsort((edge_src, tl_e, blk_id, chunk, core))
        core_s, tl_s, blk_s = core[order], tl_e[order], blk_id[order]
        src_s, row_s, val_s = edge_src[order], row_e[order], edge_val[order]

        tb_key = (core_s * tpc + tl_s) * nblk + blk_s
        n_tb = np.bincount(tb_key, minlength=ncr * tpc * nblk).reshape(ncr, tpc, nblk)
        g = -(-n_tb.max(0) // P)               # [tpc, nblk]
        self.g = g

        E = len(tb_key)
        change = np.r_[True, tb_key[1:] != tb_key[:-1]] if E else np.array([], bool)
        starts = np.flatnonzero(change)
        sizes = np.diff(np.r_[starts, E])
        rank = np.arange(E) - np.repeat(starts, sizes)

        # static layout: chunk -> block -> tile -> g[t,b]*128 slots
        base = np.zeros((tpc, nblk), np.int64)
        self.chunks = []
        slot = 0
        gidx = 0
        coff = 0
        for i in range(nch):
            tiles = list(range(i * ct, min((i + 1) * ct, tpc)))
            ch = dict(tiles=tiles, gchunk0=gidx, gb0=[], segG=[], coff=[],
                      tile_ops=[[] for _ in tiles])
            g0 = gidx
            for b in range(nblk):
                ch["gb0"].append(gidx - g0)
                segG = 0
                for tl, t in enumerate(tiles):
                    gtb = int(g[t, b])
                    base[t, b] = slot
                    if gtb:
                        ch["tile_ops"][tl].append((b, segG, segG + gtb))
                    segG += gtb
                    slot += gtb * P
                ch["segG"].append(segG)
                ch["coff"].append(coff)
                coff += 8 * segG
                gidx += segG
            ch["Gc"] = gidx - g0
            ch["split"] = self._pick_split(ch)
            self.chunks.append(ch)
        self.GT = gidx
        self.TOT = slot
        self.ICOLS = coff
        self.Gc_max = max(ch["Gc"] for ch in self.chunks)

        gslot = core_s * self.TOT + base[tl_s, blk_s] + rank
        idx_flat = np.zeros(ncr * self.TOT, np.int16)
        val_flat = np.zeros(ncr * self.TOT, np.float32)
        dst_flat = np.zeros(ncr * self.TOT, np.float32)
        idx_flat[gslot] = (src_s % cfg.blk).astype(np.int16)
        val_flat[gslot] = val_s
        dst_flat[gslot] = row_s.astype(np.float32)

        self.val_w = np.ascontiguousarray(
            val_flat.reshape(ncr, self.GT, P).transpose(0, 2, 1))
        self.dst_w = np.ascontiguousarray(
            dst_flat.reshape(ncr, self.GT, P).transpose(0, 2, 1))

        ir = idx_flat.reshape(ncr, self.TOT)
        segs = []
        s0 = 0
        for ch in self.chunks:
            for b in range(nblk):
                L = ch["segG"][b] * P
                if L == 0:
                    continue
                seg = ir[:, s0:s0 + L].reshape(ncr, L // 16, 16).transpose(0, 2, 1)
                segs.append(np.tile(seg, (1, 8, 1)))
                s0 += L
        self.idx_w = (np.concatenate(segs, axis=2) if segs
                      else np.zeros((ncr, P, 0), np.int16))
        assert self.idx_w.shape == (ncr, P, self.ICOLS)
        self.n_matmuls = sum(hi - lo for ch in self.chunks
                             for ops in ch["tile_ops"] for (_, lo, hi) in ops)

    @staticmethod
    def _pick_split(ch):
        """Split point (group index within chunk) at a (tile,block)-run
        boundary nearest Gc/2, for the two S-slab mask ops."""
        bounds = set([0, ch["Gc"]])
        for b, gb0 in enumerate(ch["gb0"]):
            for ops in ch["tile_ops"]:
                for (bb, lo, hi) in ops:
                    if bb == b:
                        bounds.add(gb0 + lo)
                        bounds.add(gb0 + hi)
        tgt = ch["Gc"] / 2
        return min(bounds, key=lambda x: abs(x - tgt))


# ---------------------------------------------------------------- kernels
def build_k1(cfg: Cfg):
    """support^T = W1^T @ x^T, node-sharded; weight-stationary.

    Host provides x^T as [kb, 128 feats, rows_pad]; kb is the OUTER loop
    within each node batch so LDWEIGHTS fires only 4x per batch instead of
    per-matmul. Output is support^T [64, rows_pad]; host re-transposes."""
    H = cfg.hidden
    nc = bacc.Bacc(None, target_bir_lowering=False)
    xt_d = nc.dram_tensor("xt", [cfg.kb, P, cfg.rows_pad], F32, kind="ExternalInput")
    w1_d = nc.dram_tensor("w1", [cfg.f_in, H], F32, kind="ExternalInput")
    sup_d = nc.dram_tensor("supT", [H, cfg.rows_pad], F32, kind="ExternalOutput")

    SL = 256              # nodes per psum slice
    NB = 8                # slices per batch (one psum bank each)
    BATCH = SL * NB       # 2048 nodes per DMA/compute batch
    nbatch = -(-cfg.rows_pad // BATCH)
    with tile.TileContext(nc) as tc:
        with (
            tc.tile_pool(name="const", bufs=1) as cpool,
            tc.tile_pool(name="xload", bufs=2) as xpool,
            tc.tile_pool(name="sout", bufs=2) as opool,
            tc.tile_pool(name="ps", bufs=1, space="PSUM") as pspool,
        ):
            w1_t = cpool.tile([P, cfg.kb, H], F32)
            nc.sync.dma_start(w1_t[:], w1_d[:].rearrange("(kb p) n -> p kb n", p=P))
            for bi in range(nbatch):
                n0 = bi * BATCH
                nn = min(BATCH, cfg.rows_pad - n0)
                nsl = -(-nn // SL)
                xsb = xpool.tile([P, cfg.kb, nn], F32, tag="xsb")
                nc.sync.dma_start(xsb[:], xt_d[:, :, n0:n0 + nn].rearrange("k p n -> p k n"))
                pss = [pspool.tile([P, SL], F32, name=f"ps{s}", tag=f"ps{s}")
                       for s in range(nsl)]
                for kb in range(cfg.kb):
                    for s in range(nsl):
                        c0 = s * SL
                        cw = min(SL, nn - c0)
                        nc.tensor.matmul(
                            pss[s][:H, :cw], w1_t[:, kb, :],
                            xsb[:, kb, c0:c0 + cw],
                            start=(kb == 0), stop=(kb == cfg.kb - 1))
                osb = opool.tile([H, nn], F32, tag="osb")
                for s in range(nsl):
                    c0 = s * SL
                    cw = min(SL, nn - c0)
                    nc.scalar.activation(osb[:, c0:c0 + cw], pss[s][:H, :cw], CPY)
                nc.sync.dma_start(sup_d[:, n0:n0 + nn], osb[:])
    nc.compile()
    return nc


def build_spmm(cfg: Cfg, sch: Sched, layer: int):
    """Per-core spmm over the full gather table.
    layer=1: +b1, relu, @W2 -> hw2 shard [rows_pad, 16].
    layer=2: table rows are (h@W2) padded to 64 cols; spmm over first 16
    cols, +b2, softmax -> out shard."""
    H, C, ct = cfg.hidden, cfg.n_class, cfg.chunk_tiles
    nc = bacc.Bacc(None, target_bir_lowering=False, num_swdge_queues=4)
    chunks, GT, ICOLS = sch.chunks, sch.GT, sch.ICOLS
    nblk, tab_rows, VW = cfg.nblk, cfg.table_rows, 1
    tab_d = nc.dram_tensor("table", [tab_rows, H], F32, kind="ExternalInput")
    idx_d = nc.dram_tensor("idx", [P, max(ICOLS, 16)], I16, kind="ExternalInput")
    dst_d = nc.dram_tensor("dstv", [P, max(GT, 1)], F32, kind="ExternalInput")
    val_d = nc.dram_tensor("valv", [P, max(GT * VW, 1)], F32, kind="ExternalInput")
    iota_d = nc.dram_tensor("iota", [P, P], F32, kind="ExternalInput")
    OUTF = C
    if layer == 1:
        b1_d = nc.dram_tensor("b1c", [H, 1], F32, kind="ExternalInput")
        w2_d = nc.dram_tensor("w2", [H, C], F32, kind="ExternalInput")
        out_d = nc.dram_tensor("hw2", [cfg.rows_pad, C], F32, kind="ExternalOutput")
        AGGF = H
    else:
        b2_d = nc.dram_tensor("b2r", [P, ct * C], F32, kind="ExternalInput")
        out_d = nc.dram_tensor("oout", [cfg.rows_pad, C], F32, kind="ExternalOutput")
        AGGF = C

    with tile.TileContext(nc) as tc:
        with (
            tc.tile_pool(name="const", bufs=1) as cpool,
            tc.tile_pool(name="gath", bufs=2) as gpool,
            tc.tile_pool(name="g16", bufs=3) as g16pool,
            tc.tile_pool(name="seg", bufs=2) as spool,
            tc.tile_pool(name="epi", bufs=2) as epool,
            tc.tile_pool(name="hsb", bufs=2) as hpool,
            tc.tile_pool(name="psA", bufs=6, space="PSUM") as psA,
            tc.tile_pool(name="psC", bufs=2, space="PSUM") as psC,
        ):
            iota_t = cpool.tile([P, P], F32)
            nc.sync.dma_start(iota_t[:], iota_d[:])
            # per-chunk idx tiles: the first gather only waits for its own
            # chunk's indices, not the whole table
            idx_ts = []
            for ci, ch in enumerate(chunks):
                c0 = ch["coff"][0]
                cw = max(sum(8 * s for s in ch["segG"]), 16)
                it = cpool.tile([P, cw], I16, name=f"idx{ci}", tag=f"idx{ci}")
                nc.sync.dma_start(it[:], idx_d[:, c0:c0 + cw])
                idx_ts.append((c0, it))
            # per-chunk dst/val tiles: the first chunk's masks and folds
            # only wait on their own slice of the tables
            dv_ts = []
            for ci, ch in enumerate(chunks):
                g0c = ch["gchunk0"]
                gw = max(ch["Gc"], 1)
                dt = cpool.tile([P, gw], F32, name=f"dst{ci}", tag=f"dst{ci}")
                vt = cpool.tile([P, gw * VW], F32, name=f"val{ci}", tag=f"val{ci}")
                nc.sync.dma_start(dt[:], dst_d[:, g0c:g0c + gw])
                nc.sync.dma_start(vt[:], val_d[:, g0c * VW:(g0c + gw) * VW])
                dv_ts.append((dt, vt))
            if layer == 1:
                b1_t = cpool.tile([H, 1], F32)
                w2_t = cpool.tile([H, C], F32)
                nc.sync.dma_start(b1_t[:], b1_d[:])
                nc.sync.dma_start(w2_t[:], w2_d[:])
            else:
                b2_t = cpool.tile([P, ct * C], F32)
                nc.sync.dma_start(b2_t[:], b2_d[:])

            ncall = 0
            for ci, ch in enumerate(chunks):
                n_t = len(ch["tiles"])
                Gc, g0, sp = ch["Gc"], ch["gchunk0"], ch["split"]
                cbase, idx_t = idx_ts[ci]
                dst_t, val_t = dv_ts[ci]
                # per-block gather tiles: each block's value-fold waits only
                # on its own gather, and the next chunk's gather on block b
                # waits only on this chunk's block-b fold (finer pipeline).
                ghi = g16pool.tile([P, max(Gc, 1), AGGF], F16, tag="ghi")
                # all S masks of the chunk in two fp16 slab ops (0/1 exact)
                slabs = []
                for (a0, a1) in ((0, sp), (sp, Gc)):
                    R = a1 - a0
                    if R <= 0:
                        slabs.append(None)
                        continue
                    st = spool.tile([P, R, P], F16, tag=f"st{0 if a0 == 0 else 1}")
                    nc.vector.tensor_tensor(
                        st[:],
                        dst_t[:, a0:a1].unsqueeze(2).broadcast_to([P, R, P]),
                        iota_t[:].unsqueeze(1).broadcast_to([P, R, P]),
                        op=EQ)
                    slabs.append((a0, st))

                for b in range(nblk):
                    segG = ch["segG"][b]
                    if segG == 0:
                        continue
                    gb0 = ch["gb0"][b]
                    gtb = gpool.tile([P, segG, H], F32, tag=f"gt{b}")
                    nc.gpsimd.dma_gather(
                        gtb[:],
                        tab_d[b * cfg.blk:(b + 1) * cfg.blk, :],
                        idx_t[:, ch["coff"][b] - cbase:
                              ch["coff"][b] - cbase + 8 * segG],
                        segG * P, segG * P, H, single_packet=False,
                        queue_num=ncall % 4)
                    ncall += 1
                    # fold edge values + fp16 cast in one DVE op (pad slots
                    # have val=0 -> contribute 0).
                    nc.vector.tensor_tensor(
                        ghi[:, gb0:gb0 + segG, :], gtb[:, :, :AGGF],
                        val_t[:, gb0:gb0 + segG]
                        .unsqueeze(2).broadcast_to([P, segG, AGGF]),
                        op=MUL)
                def s_slice(k):
                    if slabs[0] is not None and k < sp:
                        a0, st = slabs[0]
                        return st[:, k - a0, :]
                    a0, st = slabs[1]
                    return st[:, k - a0, :]

                hsb = hpool.tile([P, n_t, OUTF], F32, tag="hsb")
                if layer == 1:
                    aT = epool.tile([H, n_t, P], F32, tag="aT")
                for tl in range(n_t):
                    ops = ch["tile_ops"][tl]
                    nmm = sum(hi - lo for (_, lo, hi) in ops)
                    k = 0
                    if layer == 1:
                        # transposed aggregate psum [feat, row] so that the
                        # per-feature bias + relu fuse into the ACT
                        # evacuation, which also feeds W2 without a transpose
                        ps = psA.tile([H, P], F32, tag="agg")
                        if not ops:
                            nc.vector.memset(ps[:], 0.0)
                        for (b, lo, hi) in ops:
                            for r in range(lo, hi):
                                kk = ch["gb0"][b] + r
                                nc.tensor.matmul(
                                    ps[:], ghi[:, kk, :], s_slice(kk),
                                    start=(k == 0), stop=(k == nmm - 1))
                                k += 1
                        nc.scalar.activation(aT[:, tl, :], ps[:], RELU,
                                             bias=b1_t[:])
                        ps3 = psC.tile([P, C], F32, tag="lg")
                        nc.tensor.matmul(ps3[:], aT[:, tl, :], w2_t[:],
                                         start=True, stop=True)
                        nc.scalar.activation(hsb[:, tl, :], ps3[:], CPY)
                    else:
                        ps = psA.tile([P, C], F32, tag="agg")
                        if not ops:
                            nc.vector.memset(ps[:], 0.0)
                        for (b, lo, hi) in ops:
                            for r in range(lo, hi):
                                kk = ch["gb0"][b] + r
                                nc.tensor.matmul(
                                    ps[:], s_slice(kk), ghi[:, kk, :],
                                    start=(k == 0), stop=(k == nmm - 1))
                                k += 1
                        nc.scalar.activation(hsb[:, tl, :], ps[:], CPY)

                if layer == 2:
                    flat = hsb[:].rearrange("p t n -> p (t n)")
                    nm = epool.tile([P, n_t], F32, tag="nm")
                    nc.vector.tensor_tensor(flat, flat, b2_t[:, :n_t * C], op=ADD)
                    nc.vector.reduce_max(nm[:], hsb[:], axis=AX, negate=True)
                    nc.vector.tensor_tensor(
                        hsb[:], hsb[:],
                        nm[:].unsqueeze(2).broadcast_to([P, n_t, C]), op=ADD)
                    nc.scalar.activation(flat, flat, EXP)
                    se = epool.tile([P, n_t], F32, tag="se")
                    nc.vector.reduce_sum(se[:], hsb[:], axis=AX)
                    ri = epool.tile([P, n_t], F32, tag="ri")
                    nc.vector.reciprocal(ri[:], se[:])
                    nc.vector.tensor_tensor(
                        hsb[:], hsb[:],
                        ri[:].unsqueeze(2).broadcast_to([P, n_t, C]), op=MUL)
                t0 = ch["tiles"][0]
                nc.sync.dma_start(
                    out_d[t0 * P:(t0 + n_t) * P].rearrange("(t p) n -> p t n", p=P),
                    hsb[:])
    nc.compile()
    return nc


# ---------------------------------------------------------------- driver
LAST_PROFILE = {}


def _run(nc, in_maps, label):
    trace = os.environ.get("GCN_PROFILE") == "1"
    t0 = time.time()
    res = bass_utils.run_bass_kernel_spmd(
        nc, in_maps, core_ids=list(range(len(in_maps))), trace=trace)
    LAST_PROFILE[label] = dict(wall_s=time.time() - t0,
                               exec_time_ns=res.exec_time_ns,
                               trace=(res.instructions_and_trace or (None, None))[1])
    return res.results


def gcn_forward(cfg: Cfg, x, edge_src, edge_dst, edge_val, W1, b1, W2, b2):
    ncores, H, C, ct = cfg.n_cores, cfg.hidden, cfg.n_class, cfg.chunk_tiles
    x = np.asarray(x, np.float32)
    W1 = np.asarray(W1, np.float32)
    b1 = np.asarray(b1, np.float32)
    W2 = np.asarray(W2, np.float32)
    b2 = np.asarray(b2, np.float32)
    edge_src = np.asarray(edge_src, np.int64)
    edge_dst = np.asarray(edge_dst, np.int64)
    edge_val = np.asarray(edge_val, np.float32)

    t0 = time.time()
    sch = Sched(cfg, edge_src, edge_dst, edge_val)
    iota = np.tile(np.arange(P, dtype=np.float32), (P, 1))
    b1c = np.ascontiguousarray(b1.reshape(H, 1))
    b2r = np.tile(b2, (P, ct))
    prep_s = time.time() - t0

    # K1
    in1 = []
    for c in range(ncores):
        xs = x[c * cfg.npc:(c + 1) * cfg.npc]
        xp = np.zeros((cfg.rows_pad, cfg.f_in), np.float32)
        xp[:cfg.npc] = xs
        xt = xp.T.reshape(cfg.kb, P, cfg.rows_pad)
        in1.append(dict(xt=np.ascontiguousarray(xt), w1=W1))
    nc1 = build_k1(cfg)
    r1 = _run(nc1, in1, "k1")

    # assemble gather table: table[global node] = support[shard row]
    table = np.zeros((cfg.table_rows, H), np.float32)
    for c in range(ncores):
        table[c * cfg.npc:(c + 1) * cfg.npc] = r1[c]["supT"].T[:cfg.npc]

    in2 = [dict(table=table, idx=_pad_idx(sch, c), dstv=_pad1(sch.dst_w, c),
                valv=_pad1(sch.val_w, c), iota=iota, b1c=b1c, w2=W2)
           for c in range(ncores)]
    nc2 = build_spmm(cfg, sch, 1)
    r2 = _run(nc2, in2, "k2")

    # table2[global node, 0:16] = (h @ W2)[node]; cols 16:64 zero-padded
    tab2 = np.zeros((cfg.table_rows, H), np.float32)
    for c in range(ncores):
        tab2[c * cfg.npc:(c + 1) * cfg.npc, :C] = r2[c]["hw2"][sch.outrow[c]]

    in3 = [dict(table=tab2, idx=_pad_idx(sch, c), dstv=_pad1(sch.dst_w, c),
                valv=_pad1(sch.val_w, c), iota=iota, b2r=b2r)
           for c in range(ncores)]
    nc3 = build_spmm(cfg, sch, 2)
    r3 = _run(nc3, in3, "k3")

    out = np.concatenate(
        [r3[c]["oout"][sch.outrow[c]] for c in range(ncores)], axis=0)
    LAST_PROFILE["prep_s"] = prep_s
    LAST_PROFILE["sched"] = dict(GT=sch.GT, slots=sch.TOT, ICOLS=sch.ICOLS,
                                 n_matmuls=sch.n_matmuls,
                                 n_edges=len(edge_src) // ncores)
    return out


def _pad_idx(sch, c):
    a = sch.idx_w[c]
    if a.shape[1] >= 16:
        return a
    p = np.zeros((P, 16), np.int16)
    p[:, :a.shape[1]] = a
    return p


def _pad1(arr, c):
    a = arr[c]
    if a.shape[1] >= 1:
        return a
    return np.zeros((P, 1), a.dtype)


def _pad1f32(arr, c):
    a = arr[c]
    if a.shape[1] >= 1:
        return a
    return np.zeros((P, 1), np.float32)


def kernel(x, edge_src, edge_dst, edge_val, W1, b1, W2, b2):
    cfg = Cfg()
    return gcn_forward(cfg, x, edge_src, edge_dst, edge_val, W1, b1, W2, b2)


# ---------------------------------------------------------------- self test
def _numpy_ref(x, es, ed, ev, W1, b1, W2, b2, n):
    def spmm(d):
        g = d[es] * ev[:, None]
        out = np.zeros((n, d.shape[1]), np.float32)
        np.add.at(out, ed, g)
        return out
    h = spmm(x @ W1) + b1
    h = np.maximum(h, 0)
    lg = spmm(h) @ W2 + b2
    e = np.exp(lg - lg.max(1, keepdims=True))
    return e / e.sum(1, keepdims=True)


def _selftest():
    cfg = Cfg(n_nodes=4096, f_in=256, hidden=64, n_class=16,
              n_cores=8, chunk_tiles=2, blk=1024)
    rng = np.random.default_rng(1)
    n_edges = 65536
    x = rng.standard_normal((cfg.n_nodes, cfg.f_in), dtype=np.float32)
    es = rng.integers(0, cfg.n_nodes, n_edges)
    ed = rng.integers(0, cfg.n_nodes, n_edges)
    ev = rng.random(n_edges, dtype=np.float32)
    W1 = rng.standard_normal((cfg.f_in, cfg.hidden), dtype=np.float32) * 0.125
    b1 = rng.standard_normal(cfg.hidden, dtype=np.float32) * 0.01
    W2 = rng.standard_normal((cfg.hidden, cfg.n_class), dtype=np.float32) * 0.25
    b2 = rng.standard_normal(cfg.n_class, dtype=np.float32) * 0.01
    act = gcn_forward(cfg, x, es, ed, ev, W1, b1, W2, b2)
    ref = _numpy_ref(x, es, ed, ev, W1, b1, W2, b2, cfg.n_nodes)
    err = np.abs(act - ref).max()
    rel = err / np.abs(ref).max()
    print(f"selftest absmax={err:.3e} relmax={rel:.3e}")
    print("profile:", LAST_PROFILE)
    assert rel < 1.5e-2, "SELFTEST FAIL"
    print("SELFTEST PASS")


if __name__ == "__main__":
    _selftest()


# revision 38
# speedup vs baseline: 1.0131x; 1.0131x over previous
"""Trainium2 Bass kernel for a 2-layer GCN forward pass (8 NeuronCores).

    h      = relu(spmm(A, x @ W1) + b1)
    out    = softmax(spmm(A, h @ W2) + b2)   with spmm(A, h) @ W2 == spmm(A, h @ W2)

Strategy (graph/data parallel over 8 cores):
  K1: node-sharded dense matmul  support = x @ W1            (per-core rows)
  host: assemble full `support` gather table from the 8 shards (pure movement)
  K2: dst-sharded spmm + bias + relu -> (h @ W2) shard       (per-core rows)
  host: assemble full `h @ W2` table (padded rows)
  K3: dst-sharded spmm(16 cols) + b2 -> softmax -> out shard

spmm per core (dst tiles of 128 rows, chunks of 5 tiles):
  * host BIN-PACKS destination nodes into tiles against per-(tile, block)
    slot QUOTAS (multiples of 128, shared across cores, sized from the
    worst core +6%): sparse src-blocks are concentrated into the LAST
    tiles (small final chunks -> short kernel tail), dense blocks get a
    balanced 640/768-style split so per-tile totals match the ~2048-edge
    supply of a 128-row tile. Slot padding ~6% vs 14% for naive packing.
  * per (chunk, src-block) one `dma_gather` (int16 indices limit the table
    view to 32768 rows -> 4 blocks) fetches 256B rows from the HBM table.
    Calls round-robin the 4 SWDGE queues BY CALL COUNT (ring drain of call
    N-1 overlaps Q7 descriptor-gen of call N) and each call stays near the
    ~2.5-3k index sweet spot; larger calls hit SWDGE ring backpressure.
    Q7 descriptor generation (~2.6ns/idx, engine-serial) is the kernel's
    critical path; idx/dst/val tables are split per chunk so the first
    gather starts immediately.
  * edge values fold into the gathered rows with one broadcast multiply
    per (chunk, block) that also casts to fp16 (pad slots have val=0 ->
    contribute 0). Single fp16 pass: rel_err ~9e-3 vs the 2e-2 gate (the
    hi+lo fp16 trick is not needed; fp32 K1 matmuls ARE needed - bf16
    there alone costs 6.6e-2).
  * segment-sum as accumulating PE matmuls. Layer 1 accumulates the
    TRANSPOSED aggregate psum[64 feats, 128 rows] = ghi.T @ S so the
    per-feature b1 + relu fuse into the single ACT evacuation (per-
    partition bias) and the result feeds W2 directly without a transpose;
    the hw2 = relu(agg+b1) @ W2 [*,16] shard is the layer-2 table (A(hW2)
    == (Ah)W2). Layer 2 aggregates psum[128 rows, 16] then fused
    bias/softmax. All S masks of a half-chunk are built by ONE DVE
    tensor_tensor(is_equal) slab (f32 in, fp16 0/1 out; broadcast-pattern
    ops run at ~1 elem/lane/cycle regardless of dtype).
  * K1 is weight-stationary (x^T tiles as rhs, W1 slabs as lhsT, kb outer)
    so LDWEIGHTS fires 4x per 2048-node batch instead of per-matmul.
"""
import os
import sys
import time

for _p in ("/opt/trn_rl_repo", "/opt/pypackages"):
    if _p not in sys.path:
        sys.path.append(_p)

import numpy as np
from concourse import bacc, mybir, tile, bass_utils

F32 = mybir.dt.float32
F16 = mybir.dt.float16
BF16 = mybir.dt.bfloat16
I16 = mybir.dt.int16
AX = mybir.AxisListType.X
EQ = mybir.AluOpType.is_equal
MUL = mybir.AluOpType.mult
ADD = mybir.AluOpType.add
EXP = mybir.ActivationFunctionType.Exp
RELU = mybir.ActivationFunctionType.Relu
CPY = mybir.ActivationFunctionType.Copy

P = 128


class Cfg:
    def __init__(self, n_nodes=100000, f_in=512, hidden=64, n_class=16,
                 n_cores=8, chunk_tiles=5, blk=32768):
        self.n_nodes, self.f_in, self.hidden, self.n_class = n_nodes, f_in, hidden, n_class
        self.n_cores, self.chunk_tiles, self.blk = n_cores, chunk_tiles, blk
        assert n_nodes % n_cores == 0
        self.npc = n_nodes // n_cores
        self.tpc = -(-self.npc // P)
        self.rows_pad = self.tpc * P
        self.nblk = -(-n_nodes // blk)
        self.table_rows = self.nblk * blk
        assert f_in % P == 0
        self.kb = f_in // P
        self.n_chunks = -(-self.tpc // chunk_tiles)


def _make_quotas(cfg, deg_all):
    """Per-(tile, block) slot quotas in multiples of 128, shared across all
    cores, sized from the worst core's per-block edge totals.

    Low-traffic blocks are concentrated into few tiles (the rest get quota
    0 -> no gather groups at all); high-traffic blocks get an even
    640/768-style split."""
    tpc, nblk = cfg.tpc, cfg.nblk
    btot = deg_all.sum(1)                     # [ncores, nblk]
    bmax = btot.max(0)                        # worst core per block
    # rows needed to host every node that has edges in block b
    nwith = (deg_all > 0).sum(1).max(0)       # [nblk]
    quotas = np.zeros((tpc, nblk), np.int64)
    # base quota per block, then spread the remainder in +128 steps onto
    # the tiles with the smallest running total so per-tile edge totals
    # stay near the ~128*avg_degree supply a full tile can deliver
    order_b = np.argsort(-bmax)
    for b in order_b:
        tot_q = (int(bmax[b] * 1.06) + 127) // 128 * 128
        if tot_q == 0:
            continue
        if tot_q <= tpc * P // 2:
            # concentrate into the LAST tiles (row-feasible): the final
            # chunks then carry fewer groups, shrinking the kernel tail
            # that runs after the last gather
            k = max(tot_q // P, (int(nwith[b]) + 110) // 111)
            k = min(k, tpc)
            per = (-(-tot_q // k) + 127) // 128 * 128 if k else 0
            quotas[tpc - k:, b] = per
        else:
            q_lo = tot_q // tpc // P * P
            quotas[:, b] = q_lo
            n_hi = (tot_q - tpc * q_lo + P - 1) // P
            for _ in range(n_hi):
                t = int(np.argmin(quotas.sum(1)))
                quotas[t, b] += P
    return quotas


def _pack_tiles(cfg, deg, quotas):
    """Greedy 4-D bin packing of one core's dst nodes into tiles against
    shared quotas.

    deg: [npc, nblk] per-node per-block in-degree. Returns (tile, row) per
    node. Rows are capped at 128 [hard]; per-(tile,block) quotas [soft]."""
    npc, tpc, nblk = cfg.npc, cfg.tpc, cfg.nblk
    used = np.zeros((tpc, nblk), np.float64)
    q = np.maximum(quotas.astype(np.float64), 1e-9)
    rows = np.zeros(tpc, np.int64)
    t_of = np.zeros(npc, np.int64)
    # scarce blocks first (their quota tiles must host those nodes), then
    # big nodes while there is still room to balance them
    w = 1.0 / np.maximum(quotas.sum(0), 1).astype(np.float64)
    score = (deg * w).sum(1) * 1e6 + deg.sum(1)
    order = np.argsort(-score, kind="stable")
    degf = deg.astype(np.float64)
    for n in order:
        d = deg[n]
        df = degf[n]
        ok = (quotas >= used + d).all(1) & (rows < P)
        # balanced fill: minimize the worst relative load (blocks + rows)
        load = np.maximum(((used + df) / q).max(1), (rows + 1) / P)
        if ok.any():
            load[~ok] = np.inf
            t = int(np.argmin(load))
        else:
            open_ = rows < P
            over = np.maximum(d - (quotas - used), 0).sum(1)
            over[~open_] = np.inf
            t = int(np.argmin(over + 0.001 * load))
        t_of[n] = t
        used[t] += d
        rows[t] += 1
    # stable row numbering within each tile
    r_of = np.zeros(npc, np.int64)
    ordn = np.argsort(t_of, kind="stable")
    tt = t_of[ordn]
    first = np.r_[True, tt[1:] != tt[:-1]]
    starts = np.flatnonzero(first)
    sizes = np.diff(np.r_[starts, npc])
    r_of[ordn] = np.arange(npc) - np.repeat(starts, sizes)
    assert r_of.max() < P
    return t_of, r_of


class Sched:
    """Static (cross-core identical) spmm schedule + per-core slot arrays."""

    def __init__(self, cfg: Cfg, edge_src, edge_dst, edge_val):
        self.cfg = cfg
        ncr, nch, nblk, ct, tpc = (cfg.n_cores, cfg.n_chunks, cfg.nblk,
                                   cfg.chunk_tiles, cfg.tpc)

        core = edge_dst // cfg.npc
        dst_l = edge_dst % cfg.npc
        blk_id = edge_src // cfg.blk

        # per-core node -> (tile, row) packing
        deg = np.zeros((ncr, cfg.npc, nblk), np.int64)
        np.add.at(deg, (core, dst_l, blk_id), 1)
        quotas = _make_quotas(cfg, deg)
        self.t_of = np.zeros((ncr, cfg.npc), np.int64)
        self.r_of = np.zeros((ncr, cfg.npc), np.int64)
        for c in range(ncr):
            self.t_of[c], self.r_of[c] = _pack_tiles(cfg, deg[c], quotas)
        # outrow[c, n_local] = row in the padded shard output
        self.outrow = self.t_of * P + self.r_of

        tl_e = self.t_of[core, dst_l]          # dst tile per edge
        row_e = self.r_of[core, dst_l]         # row within tile per edge
        chunk = tl_e // ct

        order = np.lexsort((edge_src, tl_e, blk_id, chunk, core))
        core_s, tl_s, blk_s = core[order], tl_e[order], blk_id[order]
        src_s, row_s, val_s = edge_src[order], row_e[order], edge_val[order]

        tb_key = (core_s * tpc + tl_s) * nblk + blk_s
        n_tb = np.bincount(tb_key, minlength=ncr * tpc * nblk).reshape(ncr, tpc, nblk)
        g = -(-n_tb.max(0) // P)               # [tpc, nblk]
        self.g = g

        E = len(tb_key)
        change = np.r_[True, tb_key[1:] != tb_key[:-1]] if E else np.array([], bool)
        starts = np.flatnonzero(change)
        sizes = np.diff(np.r_[starts, E])
        rank = np.arange(E) - np.repeat(starts, sizes)

        # static layout: chunk -> block -> tile -> g[t,b]*128 slots
        base = np.zeros((tpc, nblk), np.int64)
        self.chunks = []
        slot = 0
        gidx = 0
        coff = 0
        for i in range(nch):
            tiles = list(range(i * ct, min((i + 1) * ct, tpc)))
            ch = dict(tiles=tiles, gchunk0=gidx, gb0=[], segG=[], coff=[],
                      tile_ops=[[] for _ in tiles])
            g0 = gidx
            for b in range(nblk):
                ch["gb0"].append(gidx - g0)
                segG = 0
                for tl, t in enumerate(tiles):
                    gtb = int(g[t, b])
                    base[t, b] = slot
                    if gtb:
                        ch["tile_ops"][tl].append((b, segG, segG + gtb))
                    segG += gtb
                    slot += gtb * P
                ch["segG"].append(segG)
                ch["coff"].append(coff)
                coff += 8 * segG
                gidx += segG
            ch["Gc"] = gidx - g0
            ch["split"] = self._pick_split(ch)
            self.chunks.append(ch)
        self.GT = gidx
        self.TOT = slot
        self.ICOLS = coff
        self.Gc_max = max(ch["Gc"] for ch in self.chunks)

        gslot = core_s * self.TOT + base[tl_s, blk_s] + rank
        idx_flat = np.zeros(ncr * self.TOT, np.int16)
        val_flat = np.zeros(ncr * self.TOT, np.float32)
        dst_flat = np.zeros(ncr * self.TOT, np.float32)
        idx_flat[gslot] = (src_s % cfg.blk).astype(np.int16)
        val_flat[gslot] = val_s
        dst_flat[gslot] = row_s.astype(np.float32)

        self.val_w = np.ascontiguousarray(
            val_flat.reshape(ncr, self.GT, P).transpose(0, 2, 1))
        self.dst_w = np.ascontiguousarray(
            dst_flat.reshape(ncr, self.GT, P).transpose(0, 2, 1))

        ir = idx_flat.reshape(ncr, self.TOT)
        segs = []
        s0 = 0
        for ch in self.chunks:
            for b in range(nblk):
                L = ch["segG"][b] * P
                if L == 0:
                    continue
                seg = ir[:, s0:s0 + L].reshape(ncr, L // 16, 16).transpose(0, 2, 1)
                segs.append(np.tile(seg, (1, 8, 1)))
                s0 += L
        self.idx_w = (np.concatenate(segs, axis=2) if segs
                      else np.zeros((ncr, P, 0), np.int16))
        assert self.idx_w.shape == (ncr, P, self.ICOLS)
        self.n_matmuls = sum(hi - lo for ch in self.chunks
                             for ops in ch["tile_ops"] for (_, lo, hi) in ops)

    @staticmethod
    def _pick_split(ch):
        """Split point (group index within chunk) at a (tile,block)-run
        boundary nearest Gc/2, for the two S-slab mask ops."""
        bounds = set([0, ch["Gc"]])
        for b, gb0 in enumerate(ch["gb0"]):
            for ops in ch["tile_ops"]:
                for (bb, lo, hi) in ops:
                    if bb == b:
                        bounds.add(gb0 + lo)
                        bounds.add(gb0 + hi)
        tgt = ch["Gc"] / 2
        return min(bounds, key=lambda x: abs(x - tgt))


# ---------------------------------------------------------------- kernels
def build_k1(cfg: Cfg):
    """support^T = W1^T @ x^T, node-sharded; weight-stationary.

    Host provides x^T as [kb, 128 feats, rows_pad]; kb is the OUTER loop
    within each node batch so LDWEIGHTS fires only 4x per batch instead of
    per-matmul. Output is support^T [64, rows_pad]; host re-transposes."""
    H = cfg.hidden
    nc = bacc.Bacc(None, target_bir_lowering=False)
    xt_d = nc.dram_tensor("xt", [cfg.kb, P, cfg.rows_pad], F32, kind="ExternalInput")
    w1_d = nc.dram_tensor("w1", [cfg.f_in, H], F32, kind="ExternalInput")
    sup_d = nc.dram_tensor("supT", [H, cfg.rows_pad], F32, kind="ExternalOutput")

    SL = 256              # nodes per psum slice
    NB = 8                # slices per batch (one psum bank each)
    BATCH = SL * NB       # 2048 nodes per DMA/compute batch
    nbatch = -(-cfg.rows_pad // BATCH)
    with tile.TileContext(nc) as tc:
        with (
            tc.tile_pool(name="const", bufs=1) as cpool,
            tc.tile_pool(name="xload", bufs=2) as xpool,
            tc.tile_pool(name="sout", bufs=2) as opool,
            tc.tile_pool(name="ps", bufs=1, space="PSUM") as pspool,
        ):
            w1_t = cpool.tile([P, cfg.kb, H], F32)
            nc.sync.dma_start(w1_t[:], w1_d[:].rearrange("(kb p) n -> p kb n", p=P))
            for bi in range(nbatch):
                n0 = bi * BATCH
                nn = min(BATCH, cfg.rows_pad - n0)
                nsl = -(-nn // SL)
                xsb = xpool.tile([P, cfg.kb, nn], F32, tag="xsb")
                nc.sync.dma_start(xsb[:], xt_d[:, :, n0:n0 + nn].rearrange("k p n -> p k n"))
                pss = [pspool.tile([P, SL], F32, name=f"ps{s}", tag=f"ps{s}")
                       for s in range(nsl)]
                for kb in range(cfg.kb):
                    for s in range(nsl):
                        c0 = s * SL
                        cw = min(SL, nn - c0)
                        nc.tensor.matmul(
                            pss[s][:H, :cw], w1_t[:, kb, :],
                            xsb[:, kb, c0:c0 + cw],
                            start=(kb == 0), stop=(kb == cfg.kb - 1))
                osb = opool.tile([H, nn], F32, tag="osb")
                for s in range(nsl):
                    c0 = s * SL
                    cw = min(SL, nn - c0)
                    nc.scalar.activation(osb[:, c0:c0 + cw], pss[s][:H, :cw], CPY)
                nc.sync.dma_start(sup_d[:, n0:n0 + nn], osb[:])
    nc.compile()
    return nc


def build_spmm(cfg: Cfg, sch: Sched, layer: int):
    """Per-core spmm over the full gather table.
    layer=1: +b1, relu, @W2 -> hw2 shard [rows_pad, 16].
    layer=2: table rows are (h@W2) padded to 64 cols; spmm over first 16
    cols, +b2, softmax -> out shard."""
    H, C, ct = cfg.hidden, cfg.n_class, cfg.chunk_tiles
    nc = bacc.Bacc(None, target_bir_lowering=False, num_swdge_queues=4)
    chunks, GT, ICOLS = sch.chunks, sch.GT, sch.ICOLS
    nblk, tab_rows, VW = cfg.nblk, cfg.table_rows, 1
    tab_d = nc.dram_tensor("table", [tab_rows, H], F32, kind="ExternalInput")
    idx_d = nc.dram_tensor("idx", [P, max(ICOLS, 16)], I16, kind="ExternalInput")
    dst_d = nc.dram_tensor("dstv", [P, max(GT, 1)], F32, kind="ExternalInput")
    val_d = nc.dram_tensor("valv", [P, max(GT * VW, 1)], F32, kind="ExternalInput")
    iota_d = nc.dram_tensor("iota", [P, P], F32, kind="ExternalInput")
    OUTF = C
    if layer == 1:
        b1_d = nc.dram_tensor("b1c", [H, 1], F32, kind="ExternalInput")
        w2_d = nc.dram_tensor("w2", [H, C], F32, kind="ExternalInput")
        out_d = nc.dram_tensor("hw2", [cfg.rows_pad, C], F32, kind="ExternalOutput")
        AGGF = H
    else:
        b2_d = nc.dram_tensor("b2r", [P, ct * C], F32, kind="ExternalInput")
        out_d = nc.dram_tensor("oout", [cfg.rows_pad, C], F32, kind="ExternalOutput")
        AGGF = C

    with tile.TileContext(nc) as tc:
        with (
            tc.tile_pool(name="const", bufs=1) as cpool,
            tc.tile_pool(name="gath", bufs=2) as gpool,
            tc.tile_pool(name="g16", bufs=3) as g16pool,
            tc.tile_pool(name="seg", bufs=2) as spool,
            tc.tile_pool(name="epi", bufs=2) as epool,
            tc.tile_pool(name="hsb", bufs=2) as hpool,
            tc.tile_pool(name="psA", bufs=6, space="PSUM") as psA,
            tc.tile_pool(name="psC", bufs=2, space="PSUM") as psC,
        ):
            iota_t = cpool.tile([P, P], F32)
            nc.sync.dma_start(iota_t[:], iota_d[:])
            # per-chunk idx tiles: the first gather only waits for its own
            # chunk's indices, not the whole table
            idx_ts = []
            for ci, ch in enumerate(chunks):
                c0 = ch["coff"][0]
                cw = max(sum(8 * s for s in ch["segG"]), 16)
                it = cpool.tile([P, cw], I16, name=f"idx{ci}", tag=f"idx{ci}")
                nc.sync.dma_start(it[:], idx_d[:, c0:c0 + cw])
                idx_ts.append((c0, it))
            # per-chunk dst/val tiles: the first chunk's masks and folds
            # only wait on their own slice of the tables
            dv_ts = []
            for ci, ch in enumerate(chunks):
                g0c = ch["gchunk0"]
                gw = max(ch["Gc"], 1)
                dt = cpool.tile([P, gw], F32, name=f"dst{ci}", tag=f"dst{ci}")
                vt = cpool.tile([P, gw * VW], F32, name=f"val{ci}", tag=f"val{ci}")
                nc.sync.dma_start(dt[:], dst_d[:, g0c:g0c + gw])
                nc.sync.dma_start(vt[:], val_d[:, g0c * VW:(g0c + gw) * VW])
                dv_ts.append((dt, vt))
            if layer == 1:
                b1_t = cpool.tile([H, 1], F32)
                w2_t = cpool.tile([H, C], F32)
                nc.sync.dma_start(b1_t[:], b1_d[:])
                nc.sync.dma_start(w2_t[:], w2_d[:])
            else:
                b2_t = cpool.tile([P, ct * C], F32)
                nc.sync.dma_start(b2_t[:], b2_d[:])

            ncall = 0
            for ci, ch in enumerate(chunks):
                n_t = len(ch["tiles"])
                Gc, g0, sp = ch["Gc"], ch["gchunk0"], ch["split"]
                cbase, idx_t = idx_ts[ci]
                dst_t, val_t = dv_ts[ci]
                # per-block gather tiles: each block's value-fold waits only
                # on its own gather, and the next chunk's gather on block b
                # waits only on this chunk's block-b fold (finer pipeline).
                ghi = g16pool.tile([P, max(Gc, 1), AGGF], F16, tag="ghi")
                # all S masks of the chunk in two fp16 slab ops (0/1 exact)
                slabs = []
                for (a0, a1) in ((0, sp), (sp, Gc)):
                    R = a1 - a0
                    if R <= 0:
                        slabs.append(None)
                        continue
                    st = spool.tile([P, R, P], F16, tag=f"st{0 if a0 == 0 else 1}")
                    nc.vector.tensor_tensor(
                        st[:],
                        dst_t[:, a0:a1].unsqueeze(2).broadcast_to([P, R, P]),
                        iota_t[:].unsqueeze(1).broadcast_to([P, R, P]),
                        op=EQ)
                    slabs.append((a0, st))

                for b in range(nblk):
                    segG = ch["segG"][b]
                    if segG == 0:
                        continue
                    gb0 = ch["gb0"][b]
                    gtb = gpool.tile([P, segG, H], F32, tag=f"gt{b}")
                    nc.gpsimd.dma_gather(
                        gtb[:],
                        tab_d[b * cfg.blk:(b + 1) * cfg.blk, :],
                        idx_t[:, ch["coff"][b] - cbase:
                              ch["coff"][b] - cbase + 8 * segG],
                        segG * P, segG * P, H, single_packet=False,
                        queue_num=ncall % 4)
                    ncall += 1
                    # fold edge values + fp16 cast in one DVE op (pad slots
                    # have val=0 -> contribute 0).
                    nc.vector.tensor_tensor(
                        ghi[:, gb0:gb0 + segG, :], gtb[:, :, :AGGF],
                        val_t[:, gb0:gb0 + segG]
                        .unsqueeze(2).broadcast_to([P, segG, AGGF]),
                        op=MUL)
                def s_slice(k):
                    if slabs[0] is not None and k < sp:
                        a0, st = slabs[0]
                        return st[:, k - a0, :]
                    a0, st = slabs[1]
                    return st[:, k - a0, :]

                hsb = hpool.tile([P, n_t, OUTF], F32, tag="hsb")
                if layer == 1:
                    aT = epool.tile([H, n_t, P], F32, tag="aT")
                for tl in range(n_t):
                    ops = ch["tile_ops"][tl]
                    nmm = sum(hi - lo for (_, lo, hi) in ops)
                    k = 0
                    if layer == 1:
                        # transposed aggregate psum [feat, row] so that the
                        # per-feature bias + relu fuse into the ACT
                        # evacuation, which also feeds W2 without a transpose
                        ps = psA.tile([H, P], F32, tag="agg")
                        if not ops:
                            nc.vector.memset(ps[:], 0.0)
                        for (b, lo, hi) in ops:
                            for r in range(lo, hi):
                                kk = ch["gb0"][b] + r
                                nc.tensor.matmul(
                                    ps[:], ghi[:, kk, :], s_slice(kk),
                                    start=(k == 0), stop=(k == nmm - 1))
                                k += 1
                        nc.scalar.activation(aT[:, tl, :], ps[:], RELU,
                                             bias=b1_t[:])
                        ps3 = psC.tile([P, C], F32, tag="lg")
                        nc.tensor.matmul(ps3[:], aT[:, tl, :], w2_t[:],
                                         start=True, stop=True)
                        nc.scalar.activation(hsb[:, tl, :], ps3[:], CPY)
                    else:
                        ps = psA.tile([P, C], F32, tag="agg")
                        if not ops:
                            nc.vector.memset(ps[:], 0.0)
                        for (b, lo, hi) in ops:
                            for r in range(lo, hi):
                                kk = ch["gb0"][b] + r
                                nc.tensor.matmul(
                                    ps[:], s_slice(kk), ghi[:, kk, :],
                                    start=(k == 0), stop=(k == nmm - 1))
                                k += 1
                        nc.scalar.activation(hsb[:, tl, :], ps[:], CPY)

                if layer == 2:
                    flat = hsb[:].rearrange("p t n -> p (t n)")
                    nm = epool.tile([P, n_t], F32, tag="nm")
                    nc.vector.tensor_tensor(flat, flat, b2_t[:, :n_t * C], op=ADD)
                    nc.vector.reduce_max(nm[:], hsb[:], axis=AX, negate=True)
                    nc.vector.tensor_tensor(
                        hsb[:], hsb[:],
                        nm[:].unsqueeze(2).broadcast_to([P, n_t, C]), op=ADD)
                    nc.scalar.activation(flat, flat, EXP)
                    se = epool.tile([P, n_t], F32, tag="se")
                    nc.vector.reduce_sum(se[:], hsb[:], axis=AX)
                    ri = epool.tile([P, n_t], F32, tag="ri")
                    nc.vector.reciprocal(ri[:], se[:])
                    nc.vector.tensor_tensor(
                        hsb[:], hsb[:],
                        ri[:].unsqueeze(2).broadcast_to([P, n_t, C]), op=MUL)
                t0 = ch["tiles"][0]
                nc.sync.dma_start(
                    out_d[t0 * P:(t0 + n_t) * P].rearrange("(t p) n -> p t n", p=P),
                    hsb[:])
    nc.compile()
    return nc


# ---------------------------------------------------------------- driver
LAST_PROFILE = {}


def _run(nc, in_maps, label):
    trace = os.environ.get("GCN_PROFILE") == "1"
    t0 = time.time()
    res = bass_utils.run_bass_kernel_spmd(
        nc, in_maps, core_ids=list(range(len(in_maps))), trace=trace)
    LAST_PROFILE[label] = dict(wall_s=time.time() - t0,
                               exec_time_ns=res.exec_time_ns,
                               trace=(res.instructions_and_trace or (None, None))[1])
    return res.results


def gcn_forward(cfg: Cfg, x, edge_src, edge_dst, edge_val, W1, b1, W2, b2):
    ncores, H, C, ct = cfg.n_cores, cfg.hidden, cfg.n_class, cfg.chunk_tiles
    x = np.asarray(x, np.float32)
    W1 = np.asarray(W1, np.float32)
    b1 = np.asarray(b1, np.float32)
    W2 = np.asarray(W2, np.float32)
    b2 = np.asarray(b2, np.float32)
    edge_src = np.asarray(edge_src, np.int64)
    edge_dst = np.asarray(edge_dst, np.int64)
    edge_val = np.asarray(edge_val, np.float32)

    t0 = time.time()
    sch = Sched(cfg, edge_src, edge_dst, edge_val)
    iota = np.tile(np.arange(P, dtype=np.float32), (P, 1))
    b1c = np.ascontiguousarray(b1.reshape(H, 1))
    b2r = np.tile(b2, (P, ct))
    prep_s = time.time() - t0

    # K1
    in1 = []
    for c in range(ncores):
        xs = x[c * cfg.npc:(c + 1) * cfg.npc]
        xp = np.zeros((cfg.rows_pad, cfg.f_in), np.float32)
        xp[:cfg.npc] = xs
        xt = xp.T.reshape(cfg.kb, P, cfg.rows_pad)
        in1.append(dict(xt=np.ascontiguousarray(xt), w1=W1))
    nc1 = build_k1(cfg)
    r1 = _run(nc1, in1, "k1")

    # assemble gather table: table[global node] = support[shard row]
    table = np.zeros((cfg.table_rows, H), np.float32)
    for c in range(ncores):
        table[c * cfg.npc:(c + 1) * cfg.npc] = r1[c]["supT"].T[:cfg.npc]

    in2 = [dict(table=table, idx=_pad_idx(sch, c), dstv=_pad1(sch.dst_w, c),
                valv=_pad1(sch.val_w, c), iota=iota, b1c=b1c, w2=W2)
           for c in range(ncores)]
    nc2 = build_spmm(cfg, sch, 1)
    r2 = _run(nc2, in2, "k2")

    # table2[global node, 0:16] = (h @ W2)[node]; cols 16:64 zero-padded
    tab2 = np.zeros((cfg.table_rows, H), np.float32)
    for c in range(ncores):
        tab2[c * cfg.npc:(c + 1) * cfg.npc, :C] = r2[c]["hw2"][sch.outrow[c]]

    in3 = [dict(table=tab2, idx=_pad_idx(sch, c), dstv=_pad1(sch.dst_w, c),
                valv=_pad1(sch.val_w, c), iota=iota, b2r=b2r)
           for c in range(ncores)]
    nc3 = build_spmm(cfg, sch, 2)
    r3 = _run(nc3, in3, "k3")

    out = np.concatenate(
        [r3[c]["oout"][sch.outrow[c]] for c in range(ncores)], axis=0)
    LAST_PROFILE["prep_s"] = prep_s
    LAST_PROFILE["sched"] = dict(GT=sch.GT, slots=sch.TOT, ICOLS=sch.ICOLS,
                                 n_matmuls=sch.n_matmuls,
                                 n_edges=len(edge_src) // ncores)
    return out


def _pad_idx(sch, c):
    a = sch.idx_w[c]
    if a.shape[1] >= 16:
        return a
    p = np.zeros((P, 16), np.int16)
    p[:, :a.shape[1]] = a
    return p


def _pad1(arr, c):
    a = arr[c]
    if a.shape[1] >= 1:
        return a
    return np.zeros((P, 1), a.dtype)


def _pad1f32(arr, c):
    a = arr[c]
    if a.shape[1] >= 1:
        return a
    return np.zeros((P, 1), np.float32)


def kernel(x, edge_src, edge_dst, edge_val, W1, b1, W2, b2):
    cfg = Cfg()
    return gcn_forward(cfg, x, edge_src, edge_dst, edge_val, W1, b1, W2, b2)


# ---------------------------------------------------------------- self test
def _numpy_ref(x, es, ed, ev, W1, b1, W2, b2, n):
    def spmm(d):
        g = d[es] * ev[:, None]
        out = np.zeros((n, d.shape[1]), np.float32)
        np.add.at(out, ed, g)
        return out
    h = spmm(x @ W1) + b1
    h = np.maximum(h, 0)
    lg = spmm(h) @ W2 + b2
    e = np.exp(lg - lg.max(1, keepdims=True))
    return e / e.sum(1, keepdims=True)


def _selftest():
    cfg = Cfg(n_nodes=4096, f_in=256, hidden=64, n_class=16,
              n_cores=8, chunk_tiles=2, blk=1024)
    rng = np.random.default_rng(1)
    n_edges = 65536
    x = rng.standard_normal((cfg.n_nodes, cfg.f_in), dtype=np.float32)
    es = rng.integers(0, cfg.n_nodes, n_edges)
    ed = rng.integers(0, cfg.n_nodes, n_edges)
    ev = rng.random(n_edges, dtype=np.float32)
    W1 = rng.standard_normal((cfg.f_in, cfg.hidden), dtype=np.float32) * 0.125
    b1 = rng.standard_normal(cfg.hidden, dtype=np.float32) * 0.01
    W2 = rng.standard_normal((cfg.hidden, cfg.n_class), dtype=np.float32) * 0.25
    b2 = rng.standard_normal(cfg.n_class, dtype=np.float32) * 0.01
    act = gcn_forward(cfg, x, es, ed, ev, W1, b1, W2, b2)
    ref = _numpy_ref(x, es, ed, ev, W1, b1, W2, b2, cfg.n_nodes)
    err = np.abs(act - ref).max()
    rel = err / np.abs(ref).max()
    print(f"selftest absmax={err:.3e} relmax={rel:.3e}")
    print("profile:", LAST_PROFILE)
    assert rel < 1.5e-2, "SELFTEST FAIL"
    print("SELFTEST PASS")


if __name__ == "__main__":
    _selftest()
